# revision 1
# baseline (speedup 1.0000x reference)
"""Trainium2 Bass kernel for nn_CrystalAttention.

Reference computation (B=8, T=2048, D=512, N=1024 neurons):
    dist[t,n]  = ||x[t] - pos[n]||                       (via x2 - 2*x.pos + p2)
    attn       = softmax_n( scales[n] / (dist + 0.1) )
    out        = (attn @ values) @ w_out.T + b_out

Sharding: data-parallel over B — core i processes batch i (2048 tokens).
All parameters replicated. No collectives.

Device kernel structure (per 128-token tile, 16 tiles/core):
  PE  : xp = xT.T @ posT        (bf16, K=512; p2 handling: see fold_p2)
  ACT : w = Ln(-2*xp + x2')     = ln ||x-pos||^2   (bias = per-token x2')
        dist = Exp(0.5*w)       = sqrt(d2)
        e = Exp(c*r) with fused row-sum (accum_out)  [c = uniform scale]
  DVE : den = dist + 0.1 ;  r = 1/den via reciprocal_approx_fast (the
        exact InstReciprocal iterates ~8 cyc/elem and measured as the
        whole kernel's bottleneck; approx is ~51 ULP, invisible here)
        final out = psum * (1/sum_e)  (tensor_scalar, per-token)
  PE  : eT = transpose(e) (f32r) ; out_psum = eT.T @ vw  (f32r matmuls,
        full-rate fp32-storage; bf16 would cost ~1e-3 output error)

Host precomputes (layout / derived-parameter prep):
  xT (bf16), x2 = sum(x^2) (f32), posT (bf16, + a -p2/2 aug row used when
  fold_p2 is off), vw = values @ w_out.T + b_out (f32).  b_out folds into
  vw exactly because softmax rows sum to 1.  When p2's spread is tiny vs
  d2 (true here: positions are 0.02-scale), mean(p2) folds into the x2
  bias and the K=1 aug matmuls are dropped (fold_p2).

All activation funcs come from the single table set
`natural_log_exp_and_others` (sqrt via exp(0.5*ln), 1/x on the DVE —
ACT Reciprocal/Rsqrt are banned for accuracy) so only one ~2.7us ACT
table load happens; the Bacc subclass pins the table-selection pass.
"""

import sys

if "/opt/trn_rl_repo" not in sys.path:
    sys.path.insert(0, "/opt/trn_rl_repo")

import numpy as np
import ml_dtypes

import bass_rust as _bass_rust
import concourse.bass as bass
import concourse.tile as tile
from concourse import bacc, mybir
from concourse.bass_utils import run_bass_kernel_spmd
from concourse.hw_specs import get_activation_tables

B, T, D = 8, 2048, 512
NN = 1024  # num_neurons used by the reference (positions[:1024])
P = 128
NTILES = T // P
NCORES = 8

F32 = mybir.dt.float32
F32R = mybir.dt.float32r
BF16 = mybir.dt.bfloat16
AF = mybir.ActivationFunctionType
ALU = mybir.AluOpType

_ACT_SET = "natural_log_exp_and_others"
_REPEAT = 1  # test-only: repeat the tile loop to measure marginal HW time
_PAIR = False  # pair adjacent tiles for the mid-chain elementwise ops
_INPLACE = True  # run the dist/den/r chain in-place in one buffer
_PROBE = 0  # test-only: 1 = skip elementwise chain (wrong numerics, perf probe)


class _PinnedBacc(bacc.Bacc):
    """Bacc whose activation-table placement only ever picks the ln/exp set.

    The stock pass picks the first table set containing each activation's
    function, which alternates natural_log <-> exp_and_others for a
    Ln;Exp;Ln;... chain (one ~2.7us table load per activation). Emptying
    every other entry (list positions are the act_func_set_id walrus uses)
    forces a single hoisted load of the combined set.
    """

    def insert_act_table_loads(self):
        has_act = any(
            isinstance(i, mybir.InstActivation)
            for b in self.main_func.blocks
            for i in b.instructions
        )
        if not has_act:
            return
        tables = list(get_activation_tables(self.m.arch).items())
        doctored = [(k, v if k == _ACT_SET else set()) for k, v in tables]
        _bass_rust.insert_act_table_loads(self, doctored)


def _build_nc(
    uniform_scale: bool, scale_c, dt_e=F32R, work_bufs: int = 4, fold_p2: bool = False
):
    """Emit the per-core program. Same program runs on all 8 cores.

    fold_p2: when the spread of p2[n]=||pos_n||^2 is negligible vs d2 (true
    for this problem's 0.02-scale positions), mean(p2) is folded into the
    per-token x2 bias on the host and the K=1 augmentation matmuls are
    dropped; the residual p2 deviation is still applied exactly via the
    posT aug row otherwise.
    """
    from contextlib import ExitStack

    nc = _PinnedBacc("TRN2", target_bir_lowering=False, debug=False)

    xT_d = nc.dram_tensor("xT", [D, T], BF16, kind="ExternalInput")
    x2_d = nc.dram_tensor("x2", [T], F32, kind="ExternalInput")
    posT_d = nc.dram_tensor("posT", [D + 1, NN], BF16, kind="ExternalInput")
    vw_d = nc.dram_tensor("vw", [NN, D], dt_e, kind="ExternalInput")
    ident_d = nc.dram_tensor("ident", [P, P], dt_e, kind="ExternalInput")
    if not uniform_scale:
        sc_d = nc.dram_tensor("sc", [NN], F32, kind="ExternalInput")
    out_d = nc.dram_tensor("out", [T, D], F32, kind="ExternalOutput")

    with tile.TileContext(nc) as tc, ExitStack() as ctx:
        consts = ctx.enter_context(tc.tile_pool(name="consts", bufs=1))
        work = ctx.enter_context(tc.tile_pool(name="work", bufs=work_bufs))
        small = ctx.enter_context(tc.tile_pool(name="small", bufs=work_bufs + 1))
        psum_xp = ctx.enter_context(tc.tile_pool(name="psum_xp", bufs=2, space="PSUM"))
        psum_e = ctx.enter_context(tc.tile_pool(name="psum_e", bufs=2, space="PSUM"))
        psum_o = ctx.enter_context(tc.tile_pool(name="psum_o", bufs=2, space="PSUM"))

        # ---- constants, loaded once; issue order favors tile-0 start ----
        x2_s = consts.tile([P, NTILES], F32)
        nc.sync.dma_start(
            out=x2_s[:], in_=x2_d.ap().rearrange("(t p) -> p t", p=P)
        )
        ident = consts.tile([P, P], dt_e)
        nc.sync.dma_start(out=ident[:], in_=ident_d.ap())
        ident_e = ident[:]
        posT_s = consts.tile([P, 4, NN], BF16)
        nc.sync.dma_start(
            out=posT_s[:], in_=posT_d.ap()[0:D].rearrange("(k p) n -> p k n", p=P)
        )
        if not fold_p2:
            augpos = consts.tile([1, NN], BF16)
            nc.sync.dma_start(out=augpos[:], in_=posT_d.ap()[D : D + 1, :])
        xT_in = xT_d.ap().rearrange("(k p) t -> p k t", p=P)
        xT_s = consts.tile([P, 4, T], BF16)
        T0 = 4 * P  # first 4 tiles' tokens land first
        nc.sync.dma_start(out=xT_s[:, :, 0:T0], in_=xT_in[:, :, 0:T0])
        vw_s = consts.tile([P, 8, D], dt_e)
        nc.sync.dma_start(
            out=vw_s[:], in_=vw_d.ap().rearrange("(j p) d -> p j d", p=P)
        )
        nc.sync.dma_start(out=xT_s[:, :, T0:T], in_=xT_in[:, :, T0:T])
        if not fold_p2:
            ones_r = consts.tile([1, P], BF16)
            nc.vector.memset(ones_r[:], 1.0)
        if not uniform_scale:
            sc_b = consts.tile([P, NN], F32)
            nc.sync.dma_start(
                out=sc_b[:],
                in_=bass.AP(tensor=sc_d.ap().tensor, offset=0, ap=[[0, P], [1, NN]]),
            )

        # ---- per-tile pipeline, tiles processed in pairs ----
        # The mid-chain elementwise ops (dist/den/r) run on a [P, 2*NN]
        # buffer covering both tiles of a pair — same element count, half
        # the per-op access overhead on ACT/DVE.
        def emit_pair(tiles):
            nh_g = len(tiles)  # tiles grouped for the mid-chain ops
            r2 = work.tile([P, nh_g, NN], F32, tag="r2")
            for hi, t in enumerate(tiles):
                tsl = slice(t * P, (t + 1) * P)
                pxp = psum_xp.tile([P, NN], F32, tag="pxp")
                for k in range(4):
                    for nh in range(2):
                        nc.tensor.matmul(
                            pxp[:, nh * 512 : (nh + 1) * 512],
                            lhsT=xT_s[:, k, tsl],
                            rhs=posT_s[:, k, nh * 512 : (nh + 1) * 512],
                            start=(k == 0),
                            stop=(k == 3 and fold_p2),
                        )
                if not fold_p2:
                    for nh in range(2):
                        nc.tensor.matmul(
                            pxp[:, nh * 512 : (nh + 1) * 512],
                            lhsT=ones_r[:],
                            rhs=augpos[:, nh * 512 : (nh + 1) * 512],
                            start=False,
                            stop=True,
                        )
                # w = ln(-2*xp' + x2) = ln ||x - pos||^2  (into pair buffer)
                if _PROBE == 1:
                    nc.scalar.mul(r2[:, hi, :], pxp[:], 1e-3)
                else:
                    nc.scalar.activation(
                        r2[:, hi, :], pxp[:], AF.Ln, bias=x2_s[:, t : t + 1], scale=-2.0
                    )
            r2f = r2[:].rearrange("p a n -> p (a n)")
            # dist = exp(0.5 w) ; den = dist + 0.1 ; r = 1/den
            if _PROBE == 1:
                pass
            elif _INPLACE:
                nc.scalar.activation(r2f, r2f, AF.Exp, scale=0.5)
                nc.vector.tensor_scalar_add(r2f, r2f, 0.1)
                # den is in [~19, ~27] here — no approx_fast edge cases; ~51
                # ULP is far below the f32r rounding already in the output.
                nc.vector.reciprocal_approx_fast(r2f, r2f)
            else:
                d2v = work.tile([P, nh_g, NN], F32, tag="d2v")
                d2f = d2v[:].rearrange("p a n -> p (a n)")
                nc.scalar.activation(d2f, r2f, AF.Exp, scale=0.5)
                nc.vector.tensor_scalar_add(d2f, d2f, 0.1)
                nc.vector.reciprocal(r2f, d2f)

            for hi, t in enumerate(tiles):
                tsl = slice(t * P, (t + 1) * P)
                # e = exp(c * r), fused row-sum
                e_t = work.tile([P, NN], dt_e, tag="e_t")
                sums = small.tile([P, 1], F32, tag="sums")
                if _PROBE == 1:
                    nc.scalar.activation(
                        e_t[:], r2[:, hi, :], AF.Copy, accum_out=sums[:]
                    )
                elif uniform_scale:
                    nc.scalar.activation(
                        e_t[:],
                        r2[:, hi, :],
                        AF.Exp,
                        scale=float(scale_c),
                        accum_out=sums[:],
                    )
                else:
                    logit_t = work.tile([P, NN], F32, tag="logit")
                    nc.vector.tensor_mul(logit_t[:], r2[:, hi, :], sc_b[:])
                    nc.scalar.activation(
                        e_t[:], logit_t[:], AF.Exp, accum_out=sums[:]
                    )
                rs = small.tile([P, 1], F32, tag="rs")
                nc.vector.reciprocal(rs[:], sums[:])

                # eT via PE transpose (f32r: 1.5 cyc/row), then to SBUF
                eT_sb = work.tile([P, NN], dt_e, tag="eT_sb")
                for h in range(2):
                    peT = psum_e.tile([P, 512], dt_e, tag="peT")
                    for j in range(4):
                        c = h * 4 + j
                        nc.tensor.transpose(
                            peT[:, j * P : (j + 1) * P],
                            e_t[:, c * P : (c + 1) * P],
                            ident_e,
                        )
                    # split PSUM->SBUF copies across ACT / DVE explicitly
                    if h == 0:
                        nc.scalar.copy(eT_sb[:, h * 512 : (h + 1) * 512], peT[:])
                    else:
                        nc.vector.tensor_copy(
                            eT_sb[:, h * 512 : (h + 1) * 512], peT[:]
                        )

                # out_u = e @ vw  (accumulate over 8 n-chunks)
                po = psum_o.tile([P, D], F32, tag="po")
                for j in range(8):
                    nc.tensor.matmul(
                        po[:],
                        lhsT=eT_sb[:, j * P : (j + 1) * P],
                        rhs=vw_s[:, j, :],
                        start=(j == 0),
                        stop=(j == 7),
                    )
                # out = out_u * (1/sum_e)   (b_out pre-folded into vw:
                # sum_n attn = 1, so attn @ (vw + 1*b_out^T) = attn@vw + b_out)
                out_t = work.tile([P, D], F32, tag="out_t")
                nc.vector.tensor_scalar_mul(out_t[:], po[:], rs[:, 0:1])
                nc.sync.dma_start(out=out_d.ap()[tsl, :], in_=out_t[:])

        for _ in range(_REPEAT):
            if _PAIR:
                for tp in range(NTILES // 2):
                    emit_pair((2 * tp, 2 * tp + 1))
            else:
                for t in range(NTILES):
                    emit_pair((t,))

    return nc


# The uniform-scale constant is data-dependent; the program is rebuilt per
# distinct value (in practice the scale is fixed so this builds once).
_CACHE: dict = {}


def _get_compiled(uniform_scale: bool, scale_c: float | None, dt_e=F32R, fold_p2=False):
    key = (uniform_scale, scale_c, dt_e, fold_p2, _REPEAT, _PAIR, _INPLACE)
    if key in _CACHE:
        return _CACHE[key]
    nc = _build_nc(uniform_scale, scale_c, dt_e=dt_e, fold_p2=fold_p2)
    nc.compile()
    _CACHE[key] = nc
    return nc


def _prep_inputs(x, positions, scales, values, w_out, b_out, dt_e=F32R):
    pos = np.asarray(positions[:NN], dtype=np.float32)
    val = np.asarray(values[:NN], dtype=np.float32)
    sc = np.asarray(scales[:NN], dtype=np.float32)
    w_out = np.asarray(w_out, dtype=np.float32)
    b_out = np.asarray(b_out, dtype=np.float32)
    x = np.asarray(x, dtype=np.float32)

    p2 = (pos.astype(np.float64) ** 2).sum(-1)
    # If p2's spread is negligible vs d2 ~ x2 + p2, fold mean(p2) into the
    # per-token bias and drop the K=1 aug matmuls (error ~spread/(2*d2) on
    # dist; far below the f32r rounding floor for this data).
    x2_scale = float(np.median((x[0].astype(np.float64) ** 2).sum(-1)))
    p2_mean = float(p2.mean())
    fold_p2 = float(p2.max() - p2.min()) < 5e-4 * (x2_scale + p2_mean)
    posT_aug = np.concatenate(
        [pos.T.astype(np.float64), (-p2 / 2)[None, :]], axis=0
    ).astype(ml_dtypes.bfloat16)  # [D+1, NN]
    # b_out folds into vw exactly: attn rows sum to 1.
    vw = (
        val.astype(np.float64) @ w_out.astype(np.float64).T
        + b_out.astype(np.float64)[None, :]
    ).astype(np.float32)

    uniform = bool(np.all(sc == sc[0]))
    scale_c = float(sc[0]) if uniform else None

    per_core = []
    for i in range(NCORES):
        xc = x[i]  # [T, D]
        x2c = (xc.astype(np.float64) ** 2).sum(-1)
        if fold_p2:
            x2c = x2c + p2_mean
        m = {
            "xT": np.ascontiguousarray(xc.T).astype(ml_dtypes.bfloat16),
            "x2": x2c.astype(np.float32),
            "posT": posT_aug,
            "vw": vw,
            "ident": np.eye(P, dtype=np.float32),
        }
        if not uniform:
            m["sc"] = sc
        per_core.append(m)
    return per_core, uniform, scale_c, fold_p2


def make_runner(nc, in_maps):
    """Persistent jitted sharded callable for repeat-timing (test utility).

    Mirrors bass2jax.run_bass_via_pjrt but without donation (this kernel
    writes every output element) and with device-resident inputs so repeated
    calls exclude host->device transfer.
    """
    import jax
    from jax.sharding import Mesh, PartitionSpec
    from jax.experimental.shard_map import shard_map
    from concourse import bass2jax

    bass2jax.install_neuronx_cc_hook()
    n_cores = len(in_maps)
    partition_name = nc.partition_id_tensor.name if nc.partition_id_tensor else None
    in_names, out_names, out_avals, zero_outs = [], [], [], []
    for alloc in nc.m.functions[0].allocations:
        if not isinstance(alloc, mybir.MemoryLocationSet):
            continue
        name = alloc.memorylocations[0].name
        if alloc.kind == "ExternalInput":
            if name != partition_name:
                in_names.append(name)
        elif alloc.kind == "ExternalOutput":
            out_names.append(name)
            shape = tuple(alloc.tensor_shape)
            dtype = mybir.dt.np(alloc.dtype)
            out_avals.append(jax.core.ShapedArray(shape, dtype))
            zero_outs.append(np.zeros(shape, dtype))
    n_params = len(in_names)
    all_names = in_names + out_names
    if partition_name is not None:
        all_names = all_names + [partition_name]

    def _body(*args):
        operands = list(args)
        if partition_name is not None:
            operands.append(bass2jax.partition_id_tensor())
        outs = bass2jax._bass_exec_p.bind(
            *operands,
            out_avals=tuple(out_avals),
            in_names=tuple(all_names),
            out_names=tuple(out_names),
            lowering_input_output_aliases=(),
            sim_require_finite=True,
            sim_require_nnan=True,
            nc=nc,
        )
        return tuple(outs)

    devices = jax.devices()[:n_cores]
    mesh = Mesh(np.asarray(devices), ("core",))
    nin = n_params + len(out_names)
    sharded = jax.jit(
        shard_map(
            _body,
            mesh=mesh,
            in_specs=(PartitionSpec("core"),) * nin,
            out_specs=(PartitionSpec("core"),) * len(out_names),
            check_rep=False,
        ),
        keep_unused=True,
    )
    concat_in = [
        np.concatenate([np.asarray(m[name]) for m in in_maps], axis=0)
        for name in in_names
    ]
    concat_zeros = [
        np.zeros((n_cores * z.shape[0], *z.shape[1:]), z.dtype) for z in zero_outs
    ]
    sharding = jax.sharding.NamedSharding(mesh, PartitionSpec("core"))
    dev_args = [jax.device_put(a, sharding) for a in concat_in + concat_zeros]
    return sharded, dev_args, out_names, out_avals


def kernel(x, positions, scales, values, w_out, b_out):
    in_maps, uniform, scale_c, fold_p2 = _prep_inputs(
        x, positions, scales, values, w_out, b_out
    )
    nc = _get_compiled(uniform, scale_c, fold_p2=fold_p2)
    res = run_bass_kernel_spmd(nc, in_maps, core_ids=list(range(NCORES)))
    out = np.stack([res.results[i]["out"] for i in range(NCORES)], axis=0)
    return out.astype(np.float32)



# revision 6
# speedup vs baseline: 1.4822x; 1.4822x over previous
"""Trainium2 Bass kernel for nn_CrystalAttention.

Reference computation (B=8, T=2048, D=512, N=1024 neurons):
    dist[t,n]  = ||x[t] - pos[n]||                       (via x2 - 2*x.pos + p2)
    attn       = softmax_n( scales[n] / (dist + 0.1) )
    out        = (attn @ values) @ w_out.T + b_out

Sharding: data-parallel over B — core i processes batch i (2048 tokens).
All parameters replicated. No collectives.

Device kernel structure (per 128-token tile, 16 tiles/core):
  PE  : xp = xT.T @ posT        (bf16, K=512; p2 handling: see fold_p2)
  ACT : w = Ln(-2*xp + x2')     = ln ||x-pos||^2   (bias = per-token x2')
        dist = Exp(0.5*w)       = sqrt(d2)
        e = Exp(c*r) with fused row-sum (accum_out)  [c = uniform scale]
  DVE : den = dist + 0.1 ;  r = 1/den via reciprocal_approx_fast (the
        exact InstReciprocal iterates ~8 cyc/elem and measured as the
        whole kernel's bottleneck; approx is ~51 ULP, invisible here)
        final out = psum * (1/sum_e)  (tensor_scalar, per-token)
  PE  : eT = transpose(e) (f32r) ; out_psum = eT.T @ vw  (f32r matmuls,
        full-rate fp32-storage; bf16 would cost ~1e-3 output error)

Host precomputes (layout / derived-parameter prep):
  xT (bf16), x2 = sum(x^2) (f32), posT (bf16, + a -p2/2 aug row used when
  fold_p2 is off), vw = values @ w_out.T + b_out (f32).  b_out folds into
  vw exactly because softmax rows sum to 1.  When p2's spread is tiny vs
  d2 (true here: positions are 0.02-scale), mean(p2) folds into the x2
  bias and the K=1 aug matmuls are dropped (fold_p2).

All activation funcs come from the single table set
`natural_log_exp_and_others` (sqrt via exp(0.5*ln), 1/x on the DVE —
ACT Reciprocal/Rsqrt are banned for accuracy) so only one ~2.7us ACT
table load happens; the Bacc subclass pins the table-selection pass.
"""

import sys

if "/opt/trn_rl_repo" not in sys.path:
    sys.path.insert(0, "/opt/trn_rl_repo")

import numpy as np
import ml_dtypes

import bass_rust as _bass_rust
import concourse.bass as bass
import concourse.tile as tile
from concourse import bacc, mybir
from concourse.bass_utils import run_bass_kernel_spmd
from concourse.hw_specs import get_activation_tables

B, T, D = 8, 2048, 512
NN = 1024  # num_neurons used by the reference (positions[:1024])
P = 128
NTILES = T // P
NCORES = 8

F32 = mybir.dt.float32
F32R = mybir.dt.float32r
BF16 = mybir.dt.bfloat16
AF = mybir.ActivationFunctionType
ALU = mybir.AluOpType

_ACT_SET = "natural_log_exp_and_others"
_REPEAT = 1  # test-only: repeat the tile loop to measure marginal HW time
_PAIR = False  # pair adjacent tiles for the mid-chain elementwise ops
_INPLACE = True  # run the dist/den/r chain in-place in one buffer
_PROBE = 0  # test-only: 1 = skip elementwise chain (wrong numerics, perf probe)


class _PinnedBacc(bacc.Bacc):
    """Bacc whose activation-table placement only ever picks the ln/exp set.

    The stock pass picks the first table set containing each activation's
    function, which alternates natural_log <-> exp_and_others for a
    Ln;Exp;Ln;... chain (one ~2.7us table load per activation). Emptying
    every other entry (list positions are the act_func_set_id walrus uses)
    forces a single hoisted load of the combined set.
    """

    def insert_act_table_loads(self):
        has_act = any(
            isinstance(i, mybir.InstActivation)
            for b in self.main_func.blocks
            for i in b.instructions
        )
        if not has_act:
            return
        tables = list(get_activation_tables(self.m.arch).items())
        doctored = [(k, v if k == _ACT_SET else set()) for k, v in tables]
        _bass_rust.insert_act_table_loads(self, doctored)


def _build_nc(
    uniform_scale: bool, scale_c, dt_e=F32R, work_bufs: int = 4, fold_p2: bool = False
):
    """Emit the per-core program. Same program runs on all 8 cores.

    fold_p2: when the spread of p2[n]=||pos_n||^2 is negligible vs d2 (true
    for this problem's 0.02-scale positions), mean(p2) is folded into the
    per-token x2 bias on the host and the K=1 augmentation matmuls are
    dropped; the residual p2 deviation is still applied exactly via the
    posT aug row otherwise.
    """
    from contextlib import ExitStack

    nc = _PinnedBacc("TRN2", target_bir_lowering=False, debug=False)

    xT_d = nc.dram_tensor("xT", [D, T], BF16, kind="ExternalInput")
    x2_d = nc.dram_tensor("x2", [T], F32, kind="ExternalInput")
    posT_d = nc.dram_tensor("posT", [D + 1, NN], BF16, kind="ExternalInput")
    vw_d = nc.dram_tensor("vw", [NN, D], dt_e, kind="ExternalInput")
    ident_d = nc.dram_tensor("ident", [P, P], dt_e, kind="ExternalInput")
    if not uniform_scale:
        sc_d = nc.dram_tensor("sc", [NN], F32, kind="ExternalInput")
    out_d = nc.dram_tensor("out", [T, D], F32, kind="ExternalOutput")

    with tile.TileContext(nc) as tc, ExitStack() as ctx:
        consts = ctx.enter_context(tc.tile_pool(name="consts", bufs=1))
        work = ctx.enter_context(tc.tile_pool(name="work", bufs=work_bufs))
        small = ctx.enter_context(tc.tile_pool(name="small", bufs=work_bufs + 1))
        psum_xp = ctx.enter_context(tc.tile_pool(name="psum_xp", bufs=2, space="PSUM"))
        psum_e = ctx.enter_context(tc.tile_pool(name="psum_e", bufs=2, space="PSUM"))
        psum_o = ctx.enter_context(tc.tile_pool(name="psum_o", bufs=2, space="PSUM"))

        # ---- constants, loaded once; issue order favors tile-0 start ----
        x2_s = consts.tile([P, NTILES], F32)
        nc.sync.dma_start(
            out=x2_s[:], in_=x2_d.ap().rearrange("(t p) -> p t", p=P)
        )
        ident = consts.tile([P, P], dt_e)
        nc.sync.dma_start(out=ident[:], in_=ident_d.ap())
        ident_e = ident[:]
        posT_s = consts.tile([P, 4, NN], BF16)
        nc.sync.dma_start(
            out=posT_s[:], in_=posT_d.ap()[0:D].rearrange("(k p) n -> p k n", p=P)
        )
        if not fold_p2:
            augpos = consts.tile([1, NN], BF16)
            nc.sync.dma_start(out=augpos[:], in_=posT_d.ap()[D : D + 1, :])
        xT_in = xT_d.ap().rearrange("(k p) t -> p k t", p=P)
        xT_s = consts.tile([P, 4, T], BF16)
        T0 = 4 * P  # first 4 tiles' tokens land first
        nc.sync.dma_start(out=xT_s[:, :, 0:T0], in_=xT_in[:, :, 0:T0])
        vw_s = consts.tile([P, 8, D], dt_e)
        nc.sync.dma_start(
            out=vw_s[:], in_=vw_d.ap().rearrange("(j p) d -> p j d", p=P)
        )
        nc.sync.dma_start(out=xT_s[:, :, T0:T], in_=xT_in[:, :, T0:T])
        if not fold_p2:
            ones_r = consts.tile([1, P], BF16)
            nc.vector.memset(ones_r[:], 1.0)
        if not uniform_scale:
            sc_b = consts.tile([P, NN], F32)
            nc.sync.dma_start(
                out=sc_b[:],
                in_=bass.AP(tensor=sc_d.ap().tensor, offset=0, ap=[[0, P], [1, NN]]),
            )

        # ---- per-tile pipeline, tiles processed in pairs ----
        # The mid-chain elementwise ops (dist/den/r) run on a [P, 2*NN]
        # buffer covering both tiles of a pair — same element count, half
        # the per-op access overhead on ACT/DVE.
        def emit_pair(tiles):
            nh_g = len(tiles)  # tiles grouped for the mid-chain ops
            r2 = work.tile([P, nh_g, NN], F32, tag="r2")
            for hi, t in enumerate(tiles):
                tsl = slice(t * P, (t + 1) * P)
                pxp = psum_xp.tile([P, NN], F32, tag="pxp")
                for k in range(4):
                    for nh in range(2):
                        nc.tensor.matmul(
                            pxp[:, nh * 512 : (nh + 1) * 512],
                            lhsT=xT_s[:, k, tsl],
                            rhs=posT_s[:, k, nh * 512 : (nh + 1) * 512],
                            start=(k == 0),
                            stop=(k == 3 and fold_p2),
                        )
                if not fold_p2:
                    for nh in range(2):
                        nc.tensor.matmul(
                            pxp[:, nh * 512 : (nh + 1) * 512],
                            lhsT=ones_r[:],
                            rhs=augpos[:, nh * 512 : (nh + 1) * 512],
                            start=False,
                            stop=True,
                        )
                # w = ln(-2*xp' + x2) = ln ||x - pos||^2  (into pair buffer)
                if _PROBE == 1:
                    nc.scalar.mul(r2[:, hi, :], pxp[:], 1e-3)
                else:
                    nc.scalar.activation(
                        r2[:, hi, :], pxp[:], AF.Ln, bias=x2_s[:, t : t + 1], scale=-2.0
                    )
            r2f = r2[:].rearrange("p a n -> p (a n)")
            # dist = exp(0.5 w) ; den = dist + 0.1 ; r = 1/den
            if _PROBE == 1:
                pass
            elif _INPLACE:
                nc.scalar.activation(r2f, r2f, AF.Exp, scale=0.5)
                nc.vector.tensor_scalar_add(r2f, r2f, 0.1)
                # den is in [~19, ~27] here — no approx_fast edge cases; ~51
                # ULP is far below the f32r rounding already in the output.
                nc.vector.reciprocal_approx_fast(r2f, r2f)
            else:
                d2v = work.tile([P, nh_g, NN], F32, tag="d2v")
                d2f = d2v[:].rearrange("p a n -> p (a n)")
                nc.scalar.activation(d2f, r2f, AF.Exp, scale=0.5)
                nc.vector.tensor_scalar_add(d2f, d2f, 0.1)
                nc.vector.reciprocal(r2f, d2f)

            for hi, t in enumerate(tiles):
                tsl = slice(t * P, (t + 1) * P)
                # e = exp(c * r), fused row-sum
                e_t = work.tile([P, NN], dt_e, tag="e_t")
                sums = small.tile([P, 1], F32, tag="sums")
                if _PROBE == 1:
                    nc.scalar.activation(
                        e_t[:], r2[:, hi, :], AF.Copy, accum_out=sums[:]
                    )
                elif uniform_scale:
                    nc.scalar.activation(
                        e_t[:],
                        r2[:, hi, :],
                        AF.Exp,
                        scale=float(scale_c),
                        accum_out=sums[:],
                    )
                else:
                    logit_t = work.tile([P, NN], F32, tag="logit")
                    nc.vector.tensor_mul(logit_t[:], r2[:, hi, :], sc_b[:])
                    nc.scalar.activation(
                        e_t[:], logit_t[:], AF.Exp, accum_out=sums[:]
                    )
                rs = small.tile([P, 1], F32, tag="rs")
                nc.vector.reciprocal(rs[:], sums[:])

                # eT via PE transpose (f32r: 1.5 cyc/row), then to SBUF
                eT_sb = work.tile([P, NN], dt_e, tag="eT_sb")
                for h in range(2):
                    peT = psum_e.tile([P, 512], dt_e, tag="peT")
                    for j in range(4):
                        c = h * 4 + j
                        nc.tensor.transpose(
                            peT[:, j * P : (j + 1) * P],
                            e_t[:, c * P : (c + 1) * P],
                            ident_e,
                        )
                    # split PSUM->SBUF copies across ACT / DVE explicitly
                    if h == 0:
                        nc.scalar.copy(eT_sb[:, h * 512 : (h + 1) * 512], peT[:])
                    else:
                        nc.vector.tensor_copy(
                            eT_sb[:, h * 512 : (h + 1) * 512], peT[:]
                        )

                # out_u = e @ vw  (accumulate over 8 n-chunks)
                po = psum_o.tile([P, D], F32, tag="po")
                for j in range(8):
                    nc.tensor.matmul(
                        po[:],
                        lhsT=eT_sb[:, j * P : (j + 1) * P],
                        rhs=vw_s[:, j, :],
                        start=(j == 0),
                        stop=(j == 7),
                    )
                # out = out_u * (1/sum_e)   (b_out pre-folded into vw:
                # sum_n attn = 1, so attn @ (vw + 1*b_out^T) = attn@vw + b_out)
                out_t = work.tile([P, D], F32, tag="out_t")
                nc.vector.tensor_scalar_mul(out_t[:], po[:], rs[:, 0:1])
                nc.sync.dma_start(out=out_d.ap()[tsl, :], in_=out_t[:])

        for _ in range(_REPEAT):
            if _PAIR:
                for tp in range(NTILES // 2):
                    emit_pair((2 * tp, 2 * tp + 1))
            else:
                for t in range(NTILES):
                    emit_pair((t,))

    return nc


FP8 = mybir.dt.float8e4
_FP8NP = ml_dtypes.float8_e4m3
DR = mybir.MatmulPerfMode.DoubleRow


def _build_nc_fast(act_scale: float, beta: float, alpha: float):
    """Fast path: near-uniform softmax regime (see _prep_fast guard).

    Per-core layout: n on partitions for phase A, tokens on partitions for
    phase B; both GEMMs fp8e4m3 DoubleRow (2 K-rows/partition). The scalar
    chain logit=sc/(dist+0.1) is linearized about the median squared
    distance (guard-verified): e = 1 + z with z = l1*(-2*x.pos) devs, so
    the whole softmax middle is ONE ACT Copy (scale) per chunk. Mean
    centering keeps the tiny n-varying signal above fp8 quantization:
      etil = alpha*z (fp8), vtil = beta*(vw - colmean(vw)) (fp8)
      out  = po*rs + C;  po = etil @ vtil (psum)
      rs   = 1/(alpha*beta*N + beta*pden); pden = sum_n etil (ones-matmul)
      C    = colmean(vw) + b_out  (exact f32 mean path; colsum(vtil)=0)
    PSUM banks are zeroed whole (2KB region) by start=True: only the first
    matmul touching a bank sets start; later disjoint halves rely on the
    pending-zero fill. pden gets a private full bank for the same reason.
    """
    from contextlib import ExitStack

    nc = bacc.Bacc("TRN2", target_bir_lowering=False, debug=False)

    TB = 512  # tokens per block
    NB = T // TB  # 4 blocks
    xpk_d = nc.dram_tensor("xpk", [P, NB * 2 * 2 * TB], FP8, kind="ExternalInput")
    ppk_d = nc.dram_tensor("ppk", [P, 2 * 2 * NN], FP8, kind="ExternalInput")
    vpk_d = nc.dram_tensor("vpk", [P, 4 * 2 * D], FP8, kind="ExternalInput")
    cm_d = nc.dram_tensor("cm", [D], F32, kind="ExternalInput")
    out_d = nc.dram_tensor("out", [T, D], F32, kind="ExternalOutput")

    with tile.TileContext(nc) as tc, ExitStack() as ctx:
        consts = ctx.enter_context(tc.tile_pool(name="consts", bufs=1))
        xin = ctx.enter_context(tc.tile_pool(name="xin", bufs=3))
        ework = ctx.enter_context(tc.tile_pool(name="ework", bufs=3))
        owork = ctx.enter_context(tc.tile_pool(name="owork", bufs=3))
        small = ctx.enter_context(tc.tile_pool(name="small", bufs=8))
        psum_a = ctx.enter_context(tc.tile_pool(name="psum_a", bufs=3, space="PSUM"))
        psum_o = ctx.enter_context(tc.tile_pool(name="psum_o", bufs=2, space="PSUM"))
        psum_d = ctx.enter_context(tc.tile_pool(name="psum_d", bufs=2, space="PSUM"))

        # consts: pos first (phase A tile-0 needs it), then vw / cm
        ps_s = consts.tile([P, 2, 2, NN], FP8)
        nc.sync.dma_start(
            out=ps_s[:], in_=ppk_d.ap().rearrange("p (c i n) -> p c i n", c=2, i=2)
        )
        vs_s = consts.tile([P, 4, 2, D], FP8)
        nc.sync.dma_start(
            out=vs_s[:], in_=vpk_d.ap().rearrange("p (j i d) -> p j i d", j=4, i=2)
        )
        cb_s = consts.tile([P, D], F32)
        nc.sync.dma_start(
            out=cb_s[:],
            in_=bass.AP(tensor=cm_d.ap().tensor, offset=0, ap=[[0, P], [1, D]]),
        )
        ones2 = consts.tile([P, 2, 1], FP8)
        nc.vector.memset(ones2[:], 1.0)

        xpk_r = xpk_d.ap().rearrange("p (b c i t) -> p b c i t", b=NB, c=2, i=2)

        def emit_block(b):
            xs = xin.tile([P, 2, 2, TB], FP8, tag="xs")
            nc.sync.dma_start(out=xs[:], in_=xpk_r[:, b, :, :, :])
            e_s = ework.tile([P, 8, TB], FP8, tag="e")
            for ncj in range(8):
                pa = psum_a.tile([P, TB], F32, tag="pa")
                nsl = slice(ncj * P, (ncj + 1) * P)
                # start/stop at bank granularity: only the first matmul
                # touching the bank starts it, only the last stops it.
                for c in range(2):
                    for th in range(2):
                        nc.tensor.matmul(
                            pa[:, th * 256 : (th + 1) * 256],
                            lhsT=ps_s[:, c, :, nsl],
                            rhs=xs[:, c, :, th * 256 : (th + 1) * 256],
                            start=(c == 0 and th == 0),
                            stop=(c == 1 and th == 1),
                            perf_mode=DR,
                        )
                nc.scalar.activation(e_s[:, ncj, :], pa[:], AF.Copy, scale=act_scale)
            for s in range(4):
                ssl = slice(s * P, (s + 1) * P)
                po = psum_o.tile([P, D], F32, tag="po")
                pd = psum_d.tile([P, 512], F32, tag="pd")  # full bank; col 0 used
                for j in range(4):
                    lhs = e_s[:, 2 * j : 2 * j + 2, ssl]
                    nc.tensor.matmul(
                        po[:, 0:256],
                        lhsT=lhs,
                        rhs=vs_s[:, j, :, 0:256],
                        start=(j == 0),
                        stop=False,
                        perf_mode=DR,
                    )
                    nc.tensor.matmul(
                        po[:, 256:512],
                        lhsT=lhs,
                        rhs=vs_s[:, j, :, 256:512],
                        start=False,
                        stop=(j == 3),
                        perf_mode=DR,
                    )
                    nc.tensor.matmul(
                        pd[:, 0:1],
                        lhsT=lhs,
                        rhs=ones2[:],
                        start=(j == 0),
                        stop=(j == 3),
                        perf_mode=DR,
                    )
                den_t = small.tile([P, 1], F32, tag="den")
                nc.vector.tensor_scalar(
                    den_t[:],
                    pd[:, 0:1],
                    float(beta),
                    float(alpha * beta * NN),
                    op0=ALU.mult,
                    op1=ALU.add,
                )
                rs_t = small.tile([P, 1], F32, tag="rs")
                nc.vector.reciprocal(rs_t[:], den_t[:])
                ot = owork.tile([P, D], F32, tag="ot")
                nc.vector.scalar_tensor_tensor(
                    ot[:],
                    po[:],
                    rs_t[:, 0:1],
                    cb_s[:],
                    op0=ALU.mult,
                    op1=ALU.add,
                )
                nc.sync.dma_start(
                    out=out_d.ap()[b * TB + s * P : b * TB + (s + 1) * P, :],
                    in_=ot[:],
                )

        for _ in range(_REPEAT):
            for b in range(NB):
                emit_block(b)

    return nc


# Programs are rebuilt per distinct derived-parameter tuple (in practice the
# data distribution is fixed so each path builds once).
_CACHE: dict = {}


def _get_compiled(key):
    full_key = key + (_REPEAT, _PAIR, _INPLACE)
    if full_key in _CACHE:
        return _CACHE[full_key]
    if key[0] == "fast":
        nc = _build_nc_fast(key[1], key[2], key[3])
    else:
        _, uniform_scale, scale_c, fold_p2 = key
        nc = _build_nc(uniform_scale, scale_c, dt_e=F32R, fold_p2=fold_p2)
    nc.compile()
    _CACHE[full_key] = nc
    return nc


def _prep_fast(x, positions, scales, values, w_out, b_out):
    """Fast-path host prep + guard. Returns (in_maps, key) or None.

    The guard simulates the device's exact arithmetic (including fp8
    quantization of every operand) on a token sample in float64 and
    compares against the exact reference math; the fast path is taken
    only when the end-to-end error is <1/10th of the correctness gate.
    """
    pos = np.asarray(positions[:NN], dtype=np.float64)
    val = np.asarray(values[:NN], dtype=np.float64)
    sc = np.asarray(scales[:NN], dtype=np.float64)
    if not np.all(sc == sc[0]):
        return None
    s0 = float(sc[0])
    w = np.asarray(w_out, dtype=np.float64)
    b = np.asarray(b_out, dtype=np.float64)
    x64 = np.asarray(x, dtype=np.float64)  # [B, T, D]

    vwb = val @ w.T  # [N, D]
    cm0 = vwb.mean(axis=0)  # [D]
    vt = vwb - cm0[None, :]
    p2 = (pos**2).sum(-1)
    x2 = (x64**2).sum(-1)  # [B, T]
    u0 = float(np.median(x2)) + float(p2.mean())
    if u0 <= 1e-6:
        return None
    q0 = np.sqrt(u0)
    l1 = -s0 * 0.5 / (q0 * (q0 + 0.1) ** 2)
    std_xp = float(np.sqrt(max(x2.mean() * p2.mean() / D, 1e-30)))
    alpha = 1.0 / max(abs(l1) * 2.0 * std_xp, 1e-30)
    amax_p = float(np.abs(pos).max())
    amax_x = float(np.abs(x64).max())
    amax_v = float(np.abs(vt).max())
    if amax_p == 0.0 or amax_v == 0.0 or amax_x == 0.0:
        return None
    sp = 16.0 / amax_p
    sx = min(1.0, 240.0 / amax_x)
    beta = 16.0 / amax_v
    act_scale = alpha * l1 * (-2.0) / (sx * sp)
    cmb = cm0 + b

    # ---- guard: device-math (with fp8 quantization) vs exact, on a sample
    BT = B * T
    idx = np.linspace(0, BT - 1, 64).astype(np.int64)
    xs = x64.reshape(BT, D)[idx]
    x2s = x2.reshape(BT)[idx]
    d2 = np.maximum(x2s[:, None] - 2.0 * xs @ pos.T + p2[None, :], 0.0)
    logit = s0 / (np.sqrt(d2) + 0.1)
    logit -= logit.max(axis=1, keepdims=True)
    ex = np.exp(logit)
    attn = ex / ex.sum(axis=1, keepdims=True)
    o_exact = attn @ vwb + b[None, :]

    f8 = lambda a: np.asarray(a, dtype=np.float32).astype(_FP8NP).astype(np.float64)
    xq = f8(sx * xs)
    pq = f8(sp * pos)
    zt = f8(act_scale * (xq @ pq.T))
    if not np.isfinite(zt).all() or np.abs(zt).max() > 440.0:
        return None
    vq = f8(beta * vt)
    po = zt @ vq
    pden = zt.sum(axis=1)
    den = alpha * beta * NN + beta * pden
    if den.min() <= 0.1 * alpha * beta * NN:
        return None
    o_fast = po / den[:, None] + cmb[None, :]
    scale_o = float(np.abs(o_exact).max())
    err = float(np.abs(o_fast - o_exact).max())
    if not np.isfinite(err) or err > 2e-3 * max(scale_o, 1e-30):
        return None

    # ---- pack per-core inputs
    TBL, NBL = 512, T // 512
    ppk = (
        np.ascontiguousarray((sp * pos).T)  # [D, N]
        .reshape(2, 2, P, NN)
        .transpose(2, 0, 1, 3)
        .reshape(P, 2 * 2 * NN)
        .astype(_FP8NP)
    )
    vpk = (
        (beta * vt).reshape(4, 2, P, D).transpose(2, 0, 1, 3).reshape(P, -1)
    ).astype(_FP8NP)
    cmb32 = cmb.astype(np.float32)
    per_core = []
    for i in range(NCORES):
        xT = np.ascontiguousarray((sx * x64[i]).T)  # [D, T]
        xpk = (
            xT.reshape(2, 2, P, NBL, TBL)
            .transpose(2, 3, 0, 1, 4)
            .reshape(P, -1)
            .astype(_FP8NP)
        )
        per_core.append({"xpk": xpk, "ppk": ppk, "vpk": vpk, "cm": cmb32})
    key = ("fast", float(act_scale), float(beta), float(alpha))
    return per_core, key


def _prep_inputs(x, positions, scales, values, w_out, b_out, dt_e=F32R):
    pos = np.asarray(positions[:NN], dtype=np.float32)
    val = np.asarray(values[:NN], dtype=np.float32)
    sc = np.asarray(scales[:NN], dtype=np.float32)
    w_out = np.asarray(w_out, dtype=np.float32)
    b_out = np.asarray(b_out, dtype=np.float32)
    x = np.asarray(x, dtype=np.float32)

    p2 = (pos.astype(np.float64) ** 2).sum(-1)
    # If p2's spread is negligible vs d2 ~ x2 + p2, fold mean(p2) into the
    # per-token bias and drop the K=1 aug matmuls (error ~spread/(2*d2) on
    # dist; far below the f32r rounding floor for this data).
    x2_scale = float(np.median((x[0].astype(np.float64) ** 2).sum(-1)))
    p2_mean = float(p2.mean())
    fold_p2 = float(p2.max() - p2.min()) < 5e-4 * (x2_scale + p2_mean)
    posT_aug = np.concatenate(
        [pos.T.astype(np.float64), (-p2 / 2)[None, :]], axis=0
    ).astype(ml_dtypes.bfloat16)  # [D+1, NN]
    # b_out folds into vw exactly: attn rows sum to 1.
    vw = (
        val.astype(np.float64) @ w_out.astype(np.float64).T
        + b_out.astype(np.float64)[None, :]
    ).astype(np.float32)

    uniform = bool(np.all(sc == sc[0]))
    scale_c = float(sc[0]) if uniform else None

    per_core = []
    for i in range(NCORES):
        xc = x[i]  # [T, D]
        x2c = (xc.astype(np.float64) ** 2).sum(-1)
        if fold_p2:
            x2c = x2c + p2_mean
        m = {
            "xT": np.ascontiguousarray(xc.T).astype(ml_dtypes.bfloat16),
            "x2": x2c.astype(np.float32),
            "posT": posT_aug,
            "vw": vw,
            "ident": np.eye(P, dtype=np.float32),
        }
        if not uniform:
            m["sc"] = sc
        per_core.append(m)
    return per_core, uniform, scale_c, fold_p2


def make_runner(nc, in_maps):
    """Persistent jitted sharded callable for repeat-timing (test utility).

    Mirrors bass2jax.run_bass_via_pjrt but without donation (this kernel
    writes every output element) and with device-resident inputs so repeated
    calls exclude host->device transfer.
    """
    import jax
    from jax.sharding import Mesh, PartitionSpec
    from jax.experimental.shard_map import shard_map
    from concourse import bass2jax

    bass2jax.install_neuronx_cc_hook()
    n_cores = len(in_maps)
    partition_name = nc.partition_id_tensor.name if nc.partition_id_tensor else None
    in_names, out_names, out_avals, zero_outs = [], [], [], []
    for alloc in nc.m.functions[0].allocations:
        if not isinstance(alloc, mybir.MemoryLocationSet):
            continue
        name = alloc.memorylocations[0].name
        if alloc.kind == "ExternalInput":
            if name != partition_name:
                in_names.append(name)
        elif alloc.kind == "ExternalOutput":
            out_names.append(name)
            shape = tuple(alloc.tensor_shape)
            dtype = mybir.dt.np(alloc.dtype)
            out_avals.append(jax.core.ShapedArray(shape, dtype))
            zero_outs.append(np.zeros(shape, dtype))
    n_params = len(in_names)
    all_names = in_names + out_names
    if partition_name is not None:
        all_names = all_names + [partition_name]

    def _body(*args):
        operands = list(args)
        if partition_name is not None:
            operands.append(bass2jax.partition_id_tensor())
        outs = bass2jax._bass_exec_p.bind(
            *operands,
            out_avals=tuple(out_avals),
            in_names=tuple(all_names),
            out_names=tuple(out_names),
            lowering_input_output_aliases=(),
            sim_require_finite=True,
            sim_require_nnan=True,
            nc=nc,
        )
        return tuple(outs)

    devices = jax.devices()[:n_cores]
    mesh = Mesh(np.asarray(devices), ("core",))
    nin = n_params + len(out_names)
    sharded = jax.jit(
        shard_map(
            _body,
            mesh=mesh,
            in_specs=(PartitionSpec("core"),) * nin,
            out_specs=(PartitionSpec("core"),) * len(out_names),
            check_rep=False,
        ),
        keep_unused=True,
    )
    concat_in = [
        np.concatenate([np.asarray(m[name]) for m in in_maps], axis=0)
        for name in in_names
    ]
    concat_zeros = [
        np.zeros((n_cores * z.shape[0], *z.shape[1:]), z.dtype) for z in zero_outs
    ]
    sharding = jax.sharding.NamedSharding(mesh, PartitionSpec("core"))
    dev_args = [jax.device_put(a, sharding) for a in concat_in + concat_zeros]
    return sharded, dev_args, out_names, out_avals


def _prep_any(x, positions, scales, values, w_out, b_out):
    """Pick the fast path when the guard passes, else the exact path."""
    fast = _prep_fast(x, positions, scales, values, w_out, b_out)
    if fast is not None:
        return fast
    in_maps, uniform, scale_c, fold_p2 = _prep_inputs(
        x, positions, scales, values, w_out, b_out
    )
    return in_maps, ("exact", uniform, scale_c, fold_p2)


def kernel(x, positions, scales, values, w_out, b_out):
    in_maps, key = _prep_any(x, positions, scales, values, w_out, b_out)
    nc = _get_compiled(key)
    res = run_bass_kernel_spmd(nc, in_maps, core_ids=list(range(NCORES)))
    out = np.stack([res.results[i]["out"] for i in range(NCORES)], axis=0)
    return out.astype(np.float32)

